# revision 1
# baseline (speedup 1.0000x reference)
"""Chamfer distance v7: v5 + pipelined u-trees (1-stage delay), DMA spread.

Flat loop over (b, g) chunks. Per chunk: t3 DMA (gpsimd) -> PE K=3 matmul
broadcast -> 2 ScalarE Squares -> DVE m2 fold -> DMA transpose (sync/scalar
alternating) -> [next chunk] -> DVE u-tree of the PREVIOUS chunk (so the
transpose latency is hidden behind a full chunk of DVE work).
"""

import sys

if "/opt/trn_rl_repo" not in sys.path:
    sys.path.insert(0, "/opt/trn_rl_repo")

import numpy as np
import ml_dtypes

import concourse.bass as bass
import concourse.tile as tile
from concourse import bacc, mybir
from concourse.bass_utils import run_bass_kernel_spmd

B = 2
N = 76800
E = 257
K = 256
NCORES = 8
NSH = N // NCORES   # 9600
P = 128
CHUNK = 1920
NG = NSH // CHUNK   # 5
NBLK = NSH // P     # 75
CBLK = CHUNK // P   # 15
SCALE = 128.0

F32 = mybir.dt.float32
F16 = mybir.dt.float16
BF16 = mybir.dt.bfloat16
MIN = mybir.AluOpType.min
ADD = mybir.AluOpType.add
AX = mybir.AxisListType


def _build_kernel(nc, tc, t3_in, e_in, dir1_out, dir2_out):
    from contextlib import ExitStack

    ctx = ExitStack()
    const_pool = ctx.enter_context(tc.tile_pool(name="const", bufs=2))
    t3_pool = ctx.enter_context(tc.tile_pool(name="t3", bufs=10))
    psum_pool = ctx.enter_context(tc.tile_pool(name="ps", bufs=2, space="PSUM"))
    d2_pool = ctx.enter_context(tc.tile_pool(name="d2", bufs=5))
    tree_pool = ctx.enter_context(tc.tile_pool(name="tree", bufs=3))
    m2_pool = ctx.enter_context(tc.tile_pool(name="m2", bufs=5))
    tp_pool = ctx.enter_context(tc.tile_pool(name="tp", bufs=5))
    acc_pool = ctx.enter_context(tc.tile_pool(name="acc", bufs=1))
    out_pool = ctx.enter_context(tc.tile_pool(name="out", bufs=2))

    ones3 = const_pool.tile([3, P], BF16, tag="ones3")
    nc.vector.memset(ones3[:], 1.0)

    negc = {}
    cmins = {}
    u3buf = {}
    for b in range(B):
        ec0 = const_pool.tile([P, 2], F32, tag=f"ec0_{b}", name=f"ec0_{b}")
        nc.gpsimd.dma_start(ec0[:], e_in[b, 0:K].rearrange("(k p) -> p k", p=P))
        ec1 = const_pool.tile([P, 2], F32, tag=f"ec1_{b}", name=f"ec1_{b}")
        nc.gpsimd.dma_start(ec1[:], e_in[b, 1 : K + 1].rearrange("(k p) -> p k", p=P))
        esum = const_pool.tile([P, 2], F32, tag=f"es_{b}", name=f"es_{b}")
        nc.vector.tensor_add(esum[:], ec0[:], ec1[:])
        negc[b] = const_pool.tile([P, 2], F32, tag=f"nc_{b}", name=f"nc_{b}")
        nc.vector.tensor_scalar_mul(negc[b][:], esum[:], -64.0)
        cmins[b] = acc_pool.tile(
            [P, 2, NG, CHUNK // 8], F16, tag=f"cm_{b}", name=f"cm_{b}"
        )
        u3buf[b] = acc_pool.tile(
            [P, NG, CBLK, 16], F16, tag=f"u3_{b}", name=f"u3_{b}"
        )

    pending = []  # (b, g, tt) entries waiting for their u-tree (2-stage lag)

    def emit_u_tree(b, g, tt):
        h = 64
        u1 = tree_pool.tile([P, CBLK, h], F16, tag="u1")
        nc.vector.tensor_tensor(
            u1[:], tt[:, :, 0:h], tt[:, :, h : 2 * h], op=MIN
        )
        h //= 2
        u2 = tree_pool.tile([P, CBLK, h], F16, tag="u2")
        nc.vector.tensor_tensor(u2[:], u1[:, :, 0:h], u1[:, :, h : 2 * h], op=MIN)
        h //= 2
        nc.vector.tensor_tensor(
            u3buf[b][:, g], u2[:, :, 0:h], u2[:, :, h : 2 * h], op=MIN
        )

    for idx in range(B * NG):
        b, g = idx % B, idx // B
        if True:
            t3sb = t3_pool.tile([3, CHUNK], BF16, tag="t3sb")
            nc.gpsimd.dma_start(t3sb[:], t3_in[b, g])
            tb = psum_pool.tile([P, CHUNK], F32, tag="tb")
            for k in range(0, CHUNK, 512):
                w = min(512, CHUNK - k)
                nc.tensor.matmul(
                    tb[:, k : k + w], ones3[:], t3sb[:, k : k + w],
                    start=True, stop=True,
                )
            d2both = d2_pool.tile([P, 2, CHUNK], F16, tag="d2both")
            for ct in range(2):
                nc.scalar.activation(
                    d2both[:, ct, :], tb[:],
                    mybir.ActivationFunctionType.Square,
                    bias=negc[b][:, ct : ct + 1],
                    scale=SCALE,
                )
            m2 = m2_pool.tile([P, CHUNK], F16, tag="m2")
            nc.vector.tensor_tensor(
                m2[:], d2both[:, 0, :], d2both[:, 1, :], op=MIN
            )
            tt = tp_pool.tile([P, CBLK, P], F16, tag="tt")
            nc.sync.dma_start_transpose(tt[:], m2[:])
            # dir1 tree (independent of the transpose -> covers its latency)
            h = CHUNK // 2
            l1 = tree_pool.tile([P, 2, h], F16, tag="l1")
            nc.vector.tensor_tensor(
                l1[:], d2both[:, :, 0:h], d2both[:, :, h : 2 * h], op=MIN
            )
            h //= 2
            l2 = tree_pool.tile([P, 2, h], F16, tag="l2")
            nc.vector.tensor_tensor(l2[:], l1[:, :, 0:h], l1[:, :, h : 2 * h], op=MIN)
            h //= 2
            nc.vector.tensor_tensor(
                cmins[b][:, :, g, :], l2[:, :, 0:h], l2[:, :, h : 2 * h], op=MIN
            )
            # u-tree of the previous chunk (its transpose has had a full
            # chunk period to land)
            pending.append((b, g, tt))
            if len(pending) > 3:
                emit_u_tree(*pending.pop(0))

    for ent in pending:
        emit_u_tree(*ent)

    for b in range(B):
        d1fin = out_pool.tile([P, 2], F32, tag="d1fin")
        nc.vector.tensor_reduce(out=d1fin[:], in_=cmins[b][:], op=MIN, axis=AX.XY)
        nc.gpsimd.dma_start(dir1_out[b].rearrange("c p -> p c"), d1fin[:])
        tmin = out_pool.tile([P, NBLK], F16, tag="tmin")
        nc.vector.tensor_reduce(
            out=tmin[:], in_=u3buf[b][:].rearrange("p g c s -> p (g c) s"),
            op=MIN, axis=AX.X,
        )
        d2sum = out_pool.tile([P, 1], F32, tag="d2sum")
        nc.vector.tensor_reduce(out=d2sum[:], in_=tmin[:], op=ADD, axis=AX.X)
        nc.gpsimd.dma_start(dir2_out[b], d2sum[:])

    ctx.close()


_CACHE = {}


def _get_compiled():
    if "nc" in _CACHE:
        return _CACHE["nc"]
    nc = bacc.Bacc(
        "TRN2",
        target_bir_lowering=False,
        debug=False,
        enable_asserts=False,
        num_devices=NCORES,
    )
    t3_in = nc.dram_tensor("t3", [B, NG, 3, CHUNK], BF16, kind="ExternalInput").ap()
    e_in = nc.dram_tensor("edges", [B, E], F32, kind="ExternalInput").ap()
    dir1_out = nc.dram_tensor("dir1", [B, 2, P], F32, kind="ExternalOutput").ap()
    dir2_out = nc.dram_tensor("dir2", [B, P, 1], F32, kind="ExternalOutput").ap()

    with tile.TileContext(nc) as tc:
        _build_kernel(nc, tc, t3_in, e_in, dir1_out, dir2_out)
    nc.compile()
    _CACHE["nc"] = nc
    return nc


def _split3(t: np.ndarray) -> np.ndarray:
    bf = ml_dtypes.bfloat16
    th = t.astype(bf)
    r1 = t - th.astype(np.float32)
    tm = r1.astype(bf)
    r2 = r1 - tm.astype(np.float32)
    tl = r2.astype(bf)
    t3 = np.stack([th, tm, tl], axis=1)
    t3 = t3.reshape(B, 3, NG, CHUNK).transpose(0, 2, 1, 3)
    return np.ascontiguousarray(t3)


def kernel(target: np.ndarray, bin_edges: np.ndarray) -> np.ndarray:
    target = np.asarray(target, dtype=np.float32)
    bin_edges = np.asarray(bin_edges, dtype=np.float32)

    t_flat = target.reshape(B, N)
    in_maps = []
    for c in range(NCORES):
        shard = t_flat[:, c * NSH : (c + 1) * NSH]
        in_maps.append({"t3": _split3(shard), "edges": bin_edges})

    nc = _get_compiled()
    res = run_bass_kernel_spmd(nc, in_maps, list(range(NCORES))).results

    dir1 = np.stack([r["dir1"] for r in res])
    dir2 = np.stack([r["dir2"] for r in res])

    per_center = dir1.min(axis=0).reshape(B, K)
    d1 = per_center.sum(axis=1, dtype=np.float64) / (SCALE * SCALE)
    d2 = dir2.sum(axis=(0, 2, 3), dtype=np.float64) / (SCALE * SCALE)
    out = np.float32((d1 + d2).mean())
    return np.asarray(out, dtype=np.float32)

